# revision 2
# baseline (speedup 1.0000x reference)
"""Trainium2 Bass kernel for GNN NodeBlock (segment_sum + MLP), 8-core SPMD.

Final (baseline 199.3us -> 63.2us measured, 3.15x):
  - Degree-sorted nodes, 512-node supergroups dealt round-robin to 8
    cores; per-supergroup slot count = roundup8(max degree) so every
    aggregation matmul is an fp8 DoubleRow (2x4-slot planes, 0.5 cyc/row).
  - Edges ship as 1-byte fp8 e4m3 with host-side per-(receiver,feature)
    error-feedback quantization (device sum exact to ~one fp8 rounding;
    end-to-end rel err 6.0e-3 vs the 2e-2 gate).
  - 4-supergroup-merged edge DMAs amortize per-entry DGE overhead; one SP-queue DMA entry feeds
four supergroups (per-entry DGE/semaphore overhead ~0.9us amortizes 4x,
raising delivery from ~1.8 to ~1.15us/supergroup so the gapless pipelined
PE stays fed). v6 was: 2-deep software pipelining of the MLP chain:
  PE stream becomes quads(i), L1(i-1), L2(i-2), quads(i+1), ... so no PE
  instruction ever waits on a just-produced ACT/DVE result (the producer
  ran a full stage earlier). Keeps the tensor engine gapless -> HAM ramps
  to full clock. Out-copies alternate ACT/DVE and are emitted two stages
  late so they never block the next supergroup's agg cast in the engine
  FIFO. Consts ride the ACT queue; SP issues only edge-tile DMAs.
"""

import os

import numpy as np
import ml_dtypes

import concourse.bacc as bacc
import concourse.mybir as mybir
import concourse.tile as tile
from concourse.bass_utils import run_bass_kernel_spmd

BF16 = ml_dtypes.bfloat16
F8 = mybir.dt.np(mybir.dt.float8e4)

N_NODES = 100000
N_CORES = 8
D = 32
SG = 512
NSGP = 25
NPCP = NSGP * SG
TOTPOS = NSGP * N_CORES * SG
BLK = N_CORES * SG

_prog_cache = {}


def _bf(x):
    return x.astype(BF16)


def _host_prep(node_attr, edge_index, edge_attr, global_attr, W1, b1, W2, b2):
    E = edge_attr.shape[0]
    recv = np.ascontiguousarray(edge_index[1]).astype(np.int64)

    deg = np.bincount(recv, minlength=N_NODES)
    sorted_ids = np.argsort(-deg, kind="stable")
    pos_of = np.empty(N_NODES, dtype=np.int64)
    pos_of[sorted_ids] = np.arange(N_NODES, dtype=np.int64)

    deg_sorted = np.concatenate(
        [deg[sorted_ids], np.zeros(TOTPOS - N_NODES, dtype=deg.dtype)]
    )
    blockmax = deg_sorted.reshape(NSGP, BLK).max(axis=1)
    KD = np.maximum(((blockmax + 7) // 8) * 8, 8).astype(np.int64)
    QF = (KD // 4).astype(np.int64)
    off = np.zeros(NSGP + 1, dtype=np.int64)
    np.cumsum(QF * SG, out=off[1:])
    TOTC = int(off[-1])

    order = np.argsort(recv, kind="stable")
    starts = np.zeros(N_NODES, dtype=np.int64)
    np.cumsum(deg[:-1], out=starts[1:])
    k = np.empty(E, dtype=np.int64)
    k[order] = np.arange(E, dtype=np.int64) - starts[recv[order]]

    # error-feedback fp8 quantization (per receiver x feature)
    ea = np.ascontiguousarray(edge_attr, dtype=np.float32)
    q = np.empty((E, D), dtype=F8)
    carry = np.zeros((N_NODES, D), dtype=np.float32)
    for kk in range(int(deg.max())):
        sel = np.flatnonzero(k == kk)
        r = recv[sel]
        v = ea[sel] + carry[r]
        qv = v.astype(F8)
        q[sel] = qv
        carry[r] = v - qv.astype(np.float32)

    P = pos_of[recv]
    i_e = P // BLK
    c_e = (P // SG) % N_CORES
    n_e = P % SG

    buf = np.zeros((N_CORES, TOTC, 4, D), dtype=F8)
    buf[c_e, off[i_e] + (k // 4) * SG + n_e, k % 4] = q
    edges_arr = np.ascontiguousarray(
        buf.reshape(N_CORES, TOTC, 4 * D).transpose(0, 2, 1)
    )

    ids_pad = np.concatenate(
        [sorted_ids, np.full(TOTPOS - N_NODES, -1, dtype=np.int64)]
    ).reshape(NSGP, N_CORES, SG)
    node_hi = _bf(node_attr.astype(np.float32))
    nodes_arr = np.zeros((N_CORES, D, NPCP), dtype=BF16)
    core_ids = []
    for c in range(N_CORES):
        idl = ids_pad[:, c, :].reshape(NPCP)
        valid = idl >= 0
        tmp = np.zeros((NPCP, D), dtype=BF16)
        tmp[valid] = node_hi[idl[valid]]
        nodes_arr[c] = tmp.T
        core_ids.append((idl, valid))

    g0 = global_attr.astype(np.float32).reshape(1, D)
    W1 = W1.astype(np.float32)
    W1a, W1b, W1c = W1[:D], W1[D : 2 * D], W1[2 * D :]
    b1p = (b1.astype(np.float32) + (g0 @ W1c).reshape(-1)).reshape(D, 1)
    w1s = np.ascontiguousarray(np.concatenate([_bf(W1a), _bf(W1b)], axis=0))
    w2s = np.ascontiguousarray(_bf(W2.astype(np.float32)))
    ident4 = np.tile(np.eye(D, dtype=F8), (4, 1))
    ident8 = np.ascontiguousarray(np.concatenate([ident4, ident4], axis=1))

    in_maps = []
    for c in range(N_CORES):
        in_maps.append(
            {
                "edges": edges_arr[c],
                "nodes": nodes_arr[c],
                "ident8": ident8,
                "w1s": w1s,
                "w2s": w2s,
                "b1p": b1p,
            }
        )
    key = tuple(int(q_) for q_ in QF)
    return in_maps, key, core_ids, b2.astype(np.float32)


def _build_program(key):
    if key in _prog_cache:
        return _prog_cache[key]

    f32 = mybir.dt.float32
    bf16 = mybir.dt.bfloat16
    f8 = mybir.dt.float8e4
    nc = bacc.Bacc(
        "TRN2", target_bir_lowering=False, debug=False, num_devices=N_CORES
    )

    QF = key
    off = [0]
    for q in QF:
        off.append(off[-1] + q * SG)
    TOTC = off[-1]
    QMAX = max(QF)

    edges_d = nc.dram_tensor("edges", [4 * D, TOTC], f8, kind="ExternalInput")
    nodes_d = nc.dram_tensor("nodes", [D, NPCP], bf16, kind="ExternalInput")
    ident8_d = nc.dram_tensor("ident8", [4 * D, 2 * D], f8, kind="ExternalInput")
    w1s_d = nc.dram_tensor("w1s", [2 * D, D], bf16, kind="ExternalInput")
    w2s_d = nc.dram_tensor("w2s", [D, D], bf16, kind="ExternalInput")
    b1p_d = nc.dram_tensor("b1p", [D, 1], f32, kind="ExternalInput")
    outT_d = nc.dram_tensor("outT", [D, NPCP], bf16, kind="ExternalOutput")

    with tile.TileContext(nc) as tc:
        with (
            tc.tile_pool(name="const", bufs=1) as cpool,
            tc.tile_pool(name="edges", bufs=4) as epool,
            tc.tile_pool(name="mlp", bufs=3) as mpool,
            tc.tile_pool(name="psA", bufs=4, space="PSUM") as pspool,
            tc.tile_pool(name="psM", bufs=2, space="PSUM") as pmpool,
            tc.tile_pool(name="psO", bufs=2, space="PSUM") as popool,
        ):
            # consts on the ACT HWDGE queue; SP's queue starts with edges
            ident8_sb = cpool.tile([4 * D, 2, D], f8)
            nc.scalar.dma_start(out=ident8_sb[:], in_=ident8_d.ap())
            w1s_sb = cpool.tile([2 * D, D], bf16)
            nc.scalar.dma_start(out=w1s_sb[:], in_=w1s_d.ap())
            w2s_sb = cpool.tile([D, D], bf16)
            nc.scalar.dma_start(out=w2s_sb[:], in_=w2s_d.ap())
            b1p_sb = cpool.tile([D, 1], f32)
            nc.scalar.dma_start(out=b1p_sb[:], in_=b1p_d.ap())
            mlpR = cpool.tile([2 * D, NPCP], bf16)
            nc.scalar.dma_start(out=mlpR[:D, :], in_=nodes_d.ap())
            outR = cpool.tile([D, NPCP], bf16)

            H1 = 13 * SG  # first writeback chunk (covers out(0..12))

            GRP = 4
            gstart = list(range(0, NSGP, GRP))
            GBMAX = max(
                sum(QF[g : g + GRP]) for g in gstart
            )
            gtiles = {}

            hts = {}

            for s in range(NSGP + 2):
                # ---- stage A: load + aggregate supergroup s ----
                if s < NSGP:
                    i = s
                    qf = QF[i]
                    ndr = qf // 2
                    if i % GRP == 0:
                        g_end = min(i + GRP, NSGP)
                        gb = (off[g_end] - off[i]) // SG
                        gt = epool.tile([4 * D, GBMAX, SG], f8, tag="et")
                        if i == 0:
                            nc.sync.dma_start(
                                out=gt[:, : qf, :],
                                in_=edges_d.ap()[:, : off[1]],
                            )
                            nc.sync.dma_start(
                                out=gt[:, qf : gb, :],
                                in_=edges_d.ap()[:, off[1] : off[g_end]],
                            )
                        elif gb > 0:
                            nc.sync.dma_start(
                                out=gt[:, :gb, :],
                                in_=edges_d.ap()[:, off[i] : off[g_end]],
                            )
                        gtiles[i // GRP] = gt
                    et = gtiles[i // GRP]
                    bb = (off[i] - off[i - i % GRP]) // SG
                    ps = pspool.tile([D, SG], f32, tag="psA")
                    nmm = ndr
                    mi = 0
                    for j in range(ndr):
                        nc.tensor.matmul(
                            out=ps[:],
                            lhsT=ident8_sb[:],
                            rhs=et[:, bb + 2 * j : bb + 2 * j + 2, :],
                            start=(mi == 0),
                            stop=(mi == nmm - 1),
                            perf_mode=mybir.MatmulPerfMode.DoubleRow,
                            skip_group_check=True,
                        )
                        mi += 1
                    nc.vector.tensor_copy(
                        out=mlpR[D:, i * SG : (i + 1) * SG], in_=ps[:]
                    )

                # ---- stage B: L1 + relu for supergroup s-1 ----
                if 0 <= s - 1 < NSGP:
                    i = s - 1
                    cols = slice(i * SG, (i + 1) * SG)
                    ph = pmpool.tile([D, SG], f32, tag="ph")
                    nc.tensor.matmul(
                        out=ph[:],
                        lhsT=w1s_sb[:],
                        rhs=mlpR[:, cols],
                        start=True,
                        stop=True,
                    )
                    ht = mpool.tile([D, SG], bf16, tag="ht")
                    nc.scalar.activation(
                        out=ht[:],
                        in_=ph[:],
                        func=mybir.ActivationFunctionType.Relu,
                        bias=b1p_sb[:],
                        scale=1.0,
                    )
                    hts[i] = ht

                # ---- stage C: L2 + out-copy for supergroup s-2 ----
                if 0 <= s - 2 < NSGP:
                    i = s - 2
                    cols = slice(i * SG, (i + 1) * SG)
                    po = popool.tile([D, SG], f32, tag="po")
                    nc.tensor.matmul(
                        out=po[:], lhsT=w2s_sb[:], rhs=hts.pop(i)[:],
                        start=True, stop=True,
                    )
                    if i % 2 == 0:
                        nc.scalar.activation(
                            out=outR[:, cols],
                            in_=po[:],
                            func=mybir.ActivationFunctionType.Copy,
                        )
                    else:
                        nc.vector.tensor_copy(out=outR[:, cols], in_=po[:])
                    if i == 12:
                        nc.scalar.dma_start(
                            out=outT_d.ap()[:, :H1], in_=outR[:, :H1]
                        )
            nc.scalar.dma_start(out=outT_d.ap()[:, H1:], in_=outR[:, H1:])

    nc.finalize()
    _prog_cache[key] = nc
    return nc


def kernel(**inputs):
    in_maps, key, core_ids, b2 = _host_prep(**inputs)
    nc = _build_program(key)
    trace = bool(os.environ.get("KERNEL_TRACE"))
    res = run_bass_kernel_spmd(nc, in_maps, list(range(N_CORES)), trace=trace)
    if trace:
        print(f"HW exec time: {res.exec_time_ns} ns")
        print(f"mean exec time: {res.mean_exec_time_ns} ns")
    out = np.empty((N_NODES, D), dtype=np.float32)
    for c in range(N_CORES):
        idl, valid = core_ids[c]
        out[idl[valid]] = res.results[c]["outT"].T.astype(np.float32)[valid]
    out += b2.reshape(1, D)
    return out
